# revision 39
# baseline (speedup 1.0000x reference)
"""Trainium2 Bass kernel for nn_Attention_60576218743412.

LayerNorm -> QKV projection -> 2D axial RoPE -> full softmax attention ->
out-projection, for x[B=4, N=2048, D=768], 12 heads of 64.

Sharding: 8 cores = 4 batches x 2 head-groups (6 heads each).  Each core
computes LN + QKV for its 6 heads, attention, and a partial out-projection
(its 384 columns of w_out); the host sums the two partials per batch.

v4 structure (from trace analysis of v1-v3):
- LN mean-subtraction is folded into host-side weight row-centering, so all
  projections run on RAW bf16 x.
- The LN scale r[t] is applied three ways so almost nothing waits on stats:
  k/v are built r-FREE from plain cos/sin tables; r_k rides the exp as a
  per-partition scale AP (and 1/r_k sits in the v "ones" column so softmax
  denominators stay unscaled); r_q is folded into per-t q-side tables.
- Attention is interleaved with the prelude: per (pr, t) the 16 key-chunk
  rounds ensure their own deps (per-t stats chain, k-build, v-build) on
  demand two rounds ahead, so exp starts as soon as x(t0) lands instead of
  after the full prelude.
- Feature-major AV with the 65th ones-column rowsum; normalize via
  reciprocal_approx_fast + DRAM-broadcast; out-projection woven into the
  next token-chunk's rounds.
"""

import numpy as np
import ml_dtypes

B, N, D = 4, 2048, 768
HEADS, DH = 12, 64
HG = 6                # heads per core
E = HG * DH           # 384: per-core qkv width
ROPE_BASE = 8192.0
LN_EPS = 1e-5
P = 128
DC = D // P           # 6 contraction chunks
ECH = E // P          # 3 e-chunks
TCH = 4               # token chunks for 512-wide matmuls
QW = N // TCH         # 512
KCH = N // P          # 16 key chunks
NPAIR = HG // 2       # head pairs per core

_GRAPH_CACHE = {}


def _build_graph():
    from contextlib import ExitStack

    import concourse.tile as tile
    from concourse import bacc, mybir

    f32 = mybir.dt.float32
    f32r = mybir.dt.float32r
    bf16 = mybir.dt.bfloat16
    AL = mybir.AluOpType
    AF = mybir.ActivationFunctionType

    nc = bacc.Bacc(None, target_bir_lowering=False)

    xT = nc.dram_tensor("xT", [TCH, P, DC, QW], bf16, kind="ExternalInput")
    wqT = nc.dram_tensor("wqT", [P, DC, E], bf16, kind="ExternalInput")
    wkT = nc.dram_tensor("wkT", [P, DC, E], bf16, kind="ExternalInput")
    wvT = nc.dram_tensor("wvT", [P, DC, E], bf16, kind="ExternalInput")
    woT = nc.dram_tensor("woT", [P, ECH, D], bf16, kind="ExternalInput")
    coordsT = nc.dram_tensor("coordsT", [2, N], f32, kind="ExternalInput")
    invf = nc.dram_tensor("invf", [P, 1], f32, kind="ExternalInput")
    permA = nc.dram_tensor("permA", [P, P], f32r, kind="ExternalInput")
    permB = nc.dram_tensor("permB", [P, P], f32r, kind="ExternalInput")
    onesb = nc.dram_tensor("onesb", [P, 1], bf16, kind="ExternalInput")
    outT = nc.dram_tensor("out", [D, N], f32, kind="ExternalOutput")

    outT_r = outT.rearrange("(c p) t -> p c t", p=P)

    MAGIC = float(2.0 ** 23)
    TWO_PI = float(2 * np.pi)
    SCALE = float(DH ** -0.5)

    with tile.TileContext(nc) as tc, ExitStack() as octx:
        consts = octx.enter_context(tc.tile_pool(name="consts", bufs=1))
        persist = octx.enter_context(tc.tile_pool(name="persist", bufs=1))
        dram = octx.enter_context(tc.tile_pool(name="dram", bufs=1, space="DRAM"))

        # PSUM: sc 2 banks x2 + av 1 bank x4 = 8 banks
        scp = octx.enter_context(tc.tile_pool(name="sc_ps", bufs=2, space="PSUM"))
        avp = octx.enter_context(tc.tile_pool(name="av_ps", bufs=3, space="PSUM"))
        bldp = octx.enter_context(tc.tile_pool(name="bld_ps", bufs=1, space="PSUM"))
        outproj_ps = [None]

        # ---------------- constants ----------------
        invf_sb = consts.tile([P, 1], f32)
        nc.scalar.dma_start(out=invf_sb[:], in_=invf[:])
        pA_sb = consts.tile([P, P], f32r)
        nc.scalar.dma_start(out=pA_sb[:], in_=permA[:])
        pB_sb = consts.tile([P, P], f32r)
        nc.scalar.dma_start(out=pB_sb[:], in_=permB[:])
        woT_sb = consts.tile([P, ECH, D], bf16)
        nc.scalar.dma_start(out=woT_sb[:], in_=woT[:])
        pi2_sb = consts.tile([P, 1], f32)
        nc.vector.memset(pi2_sb[:], float(np.pi / 2))
        onesb_sb = consts.tile([P, 1], bf16)
        nc.scalar.dma_start(out=onesb_sb[:], in_=onesb[:])

        # persistent state
        xn_sb = persist.tile([P, TCH, DC, QW], bf16)  # raw x, [t, dc, q]
        wq_sb = persist.tile([P, DC, E], bf16, tag="wq")
        wk_sb = persist.tile([P, DC, E], bf16, tag="wk")
        nc.gpsimd.dma_start(out=wk_sb[:], in_=wkT[:])
        wv_sb = persist.tile([P, DC, E], bf16, tag="wv")
        nc.gpsimd.dma_start(out=wv_sb[:], in_=wvT[:])
        kr_sb = persist.tile([P, ECH, N], bf16)     # rotated k (r-free)
        v_sb = persist.tile([P, KCH, HG * 65], bf16)  # raw v | 1/r_k col
        costab = persist.tile([P, N], f32)          # plain cos
        sintab = persist.tile([P, N], f32)          # plain sin
        qcos = persist.tile([P, N], f32)            # r_q * cos (per-t filled)
        qsin = persist.tile([P, N], f32)            # r_q * sin
        r_tok = persist.tile([P, KCH], f32)         # r, token-major
        rsc = persist.tile([P, KCH], f32)           # r * dh^-0.5 (exp scale)

        sums_d = dram.tile([TCH, 2 * QW], f32)   # per t: [sum | sumsq]
        r_d = dram.tile([1, N], f32)
        scr_d = dram.tile([TCH * NPAIR, 2 * QW], f32)

        # ones columns of v (become 1/r_k once stats land)
        for h in range(HG):
            nc.gpsimd.dma_start(
                out=v_sb[:, :, h * 65 + 64: h * 65 + 65],
                in_=onesb[:, 0:1][:, :, None].to_broadcast((P, KCH, 1)))

        # ---------------- RoPE trig tables (plain) ----------------
        with ExitStack() as ptab:
            tblp = ptab.enter_context(tc.tile_pool(name="tbl", bufs=1))
            ftab = tblp.tile([P, N], f32, name="ftab")
            for blk in range(4):
                axis = blk % 2
                nc.sync.dma_start(
                    out=ftab[32 * blk: 32 * blk + 32, :],
                    in_=coordsT[axis: axis + 1, :].to_broadcast((32, N)),
                )
            nc.vector.tensor_scalar_mul(ftab[:], ftab[:], invf_sb[:])
            # round-to-nearest via +-2^23; costab doubles as the scratch
            nc.vector.tensor_scalar(
                costab[:], ftab[:], 1.0 / TWO_PI, MAGIC, AL.mult, AL.add)
            nc.vector.tensor_scalar_sub(costab[:], costab[:], MAGIC)
            nc.vector.scalar_tensor_tensor(
                sintab[:], costab[:], -TWO_PI, ftab[:], AL.mult, AL.add)
            nc.scalar.activation(sintab[:], sintab[:], AF.Sin)
            nc.vector.tensor_scalar(
                costab[:], ftab[:], 1.0 / TWO_PI, 0.25, AL.mult, AL.add)
            nc.vector.tensor_scalar_add(costab[:], costab[:], MAGIC)
            nc.vector.tensor_scalar_sub(costab[:], costab[:], MAGIC)
            nc.vector.scalar_tensor_tensor(
                costab[:], costab[:], -TWO_PI, ftab[:], AL.mult, AL.add)
            nc.scalar.activation(costab[:], costab[:], AF.Sin, bias=pi2_sb[:])

        # working pools (created after the table scratch is released)
        rawp = octx.enter_context(tc.tile_pool(name="raw", bufs=2))
        cmbp = octx.enter_context(tc.tile_pool(name="cmb", bufs=1))
        ptp = octx.enter_context(tc.tile_pool(name="pt", bufs=2))
        qrp = octx.enter_context(tc.tile_pool(name="qr", bufs=2))
        attf = octx.enter_context(tc.tile_pool(name="attf", bufs=2))
        rsp = octx.enter_context(tc.tile_pool(name="rsp", bufs=2))
        repp = octx.enter_context(tc.tile_pool(name="repp", bufs=2))
        ostg = octx.enter_context(tc.tile_pool(name="ostg", bufs=2))
        rrep = octx.enter_context(tc.tile_pool(name="rrep", bufs=1))
        sqp = octx.enter_context(tc.tile_pool(name="xsq", bufs=2))
        stcp = octx.enter_context(tc.tile_pool(name="stc", bufs=1))
        smallp = octx.enter_context(tc.tile_pool(name="small", bufs=1))

        # x loads; wq last on sync (needed latest)
        for t, eng in zip(range(TCH), (nc.sync, nc.gpsimd, nc.sync, nc.gpsimd)):
            eng.dma_start(out=xn_sb[:, t], in_=xT[t])
        nc.sync.dma_start(out=wq_sb[:], in_=wqT[:])

        stT = smallp.tile([P, 2, KCH], f32)
        mu = smallp.tile([P, KCH], f32)
        var = smallp.tile([P, KCH], f32)
        musq = smallp.tile([P, KCH], f32)
        sdev = smallp.tile([P, KCH], f32)

        # ---------------- on-demand emission machinery ----------------
        done = set()

        def ensure(key, fn):
            if key not in done:
                done.add(key)
                fn()

        def emit_stats(t):
            # LN sums for token chunk t (two single-bank passes on bld ring)
            st = bldp.tile([P, QW], f32, space="PSUM", tag="bld", name="st")
            for dc in range(DC):
                nc.tensor.matmul(
                    st[0:1, :], onesb_sb[:], xn_sb[:, t, dc, :],
                    start=(dc == 0), stop=(dc == DC - 1))
            stc = stcp.tile([1, QW], f32, tag="stc", name="stc")
            nc.vector.tensor_copy(out=stc[:], in_=st[0:1, :])
            nc.scalar.dma_start(out=sums_d[t:t + 1, 0:QW], in_=stc[:])
            st2 = bldp.tile([P, QW], f32, space="PSUM", tag="bld", name="st2")
            for dc in range(DC):
                xsq = sqp.tile([P, QW], bf16, name="xsq")
                nc.vector.tensor_mul(xsq[:], xn_sb[:, t, dc, :], xn_sb[:, t, dc, :])
                nc.tensor.matmul(
                    st2[0:1, :], onesb_sb[:], xsq[:],
                    start=(dc == 0), stop=(dc == DC - 1))
            stc2 = stcp.tile([1, QW], f32, tag="stc", name="stc2")
            nc.vector.tensor_copy(out=stc2[:], in_=st2[0:1, :])
            nc.scalar.dma_start(out=sums_d[t:t + 1, QW:2 * QW], in_=stc2[:])

        def emit_chain(t):
            # stats -> r(t) -> exp-scale rsc, q-tables, v 1/r columns for t
            ensure(("stats", t), lambda: emit_stats(t))
            ksl4 = slice(t * 4, (t + 1) * 4)
            for s in range(2):
                nc.scalar.dma_start(
                    out=stT[:, s, ksl4],
                    in_=sums_d[t:t + 1, s * QW:(s + 1) * QW].rearrange(
                        "o (c p) -> p (o c)", p=P))
            nc.vector.tensor_scalar_mul(mu[:, ksl4], stT[:, 0, ksl4], 1.0 / D)
            nc.vector.tensor_scalar(
                var[:, ksl4], stT[:, 1, ksl4], 1.0 / D, float(LN_EPS),
                AL.mult, AL.add)
            nc.vector.tensor_mul(musq[:, ksl4], mu[:, ksl4], mu[:, ksl4])
            nc.vector.tensor_sub(var[:, ksl4], var[:, ksl4], musq[:, ksl4])
            nc.scalar.activation(sdev[:, ksl4], var[:, ksl4], AF.Sqrt)
            nc.vector.reciprocal(r_tok[:, ksl4], sdev[:, ksl4])
            nc.vector.tensor_scalar_mul(rsc[:, ksl4], r_tok[:, ksl4], SCALE)
            # q-side tables for chunk t: qcos = r * cos, qsin = r * sin
            tsl = slice(t * QW, (t + 1) * QW)
            nc.scalar.dma_start(
                out=r_d[0:1, tsl].rearrange("o (c p) -> p (o c)", p=P),
                in_=r_tok[:, ksl4])
            rr = rrep.tile([P, QW], f32, name="rr")
            nc.scalar.dma_start(out=rr[:], in_=r_d[0:1, tsl].to_broadcast((P, QW)))
            nc.vector.tensor_mul(qcos[:, tsl], costab[:, tsl], rr[:])
            nc.vector.tensor_mul(qsin[:, tsl], sintab[:, tsl], rr[:])

        def build_ops(w_sb, dst, ec, t, ctab, stab):
            """Coarse micro-ops for one projected+rotated [128, QW] tile."""
            tsl = slice(t * QW, (t + 1) * QW)
            esl = slice(ec * P, (ec + 1) * P)
            state = {}

            def op_proj():
                state["pj"] = bldp.tile([P, QW], f32, space="PSUM",
                                        tag="bld", name="pj")
                for dc in range(DC):
                    nc.tensor.matmul(
                        state["pj"][:], w_sb[:, dc, esl], xn_sb[:, t, dc, :],
                        start=(dc == 0), stop=(dc == DC - 1))

            def op_raw():
                state["raw"] = rawp.tile([P, QW], f32r, name="raw")
                nc.vector.tensor_copy(out=state["raw"][:], in_=state["pj"][:])

            def op_ep():
                state["ep"] = bldp.tile([P, QW], f32, space="PSUM",
                                        tag="bld", name="ep")
                nc.tensor.matmul(state["ep"][:], pA_sb[:], state["raw"][:],
                                 start=True, stop=True)

            def op_t1():
                state["t1"] = cmbp.tile([P, QW], f32, tag="t1", name="t1")
                nc.vector.tensor_mul(state["t1"][:], state["ep"][:],
                                     ctab[:, tsl])

            def op_op():
                state["op"] = bldp.tile([P, QW], f32, space="PSUM",
                                        tag="bld", name="opm")
                nc.tensor.matmul(state["op"][:], pB_sb[:], state["raw"][:],
                                 start=True, stop=True)

            def op_t2():
                state["t2"] = cmbp.tile([P, QW], f32, tag="t2", name="t2")
                nc.vector.tensor_mul(state["t2"][:], state["op"][:],
                                     stab[:, tsl])

            def op_add():
                nc.gpsimd.tensor_add(dst[:, ec, :] if dst.shape[2] == QW
                                     else dst[:, ec, tsl],
                                     state["t1"][:], state["t2"][:])

            return [op_proj, op_raw, op_ep, op_t1, op_op, op_t2, op_add]

        def emit_kbuild(ec, t):
            for op in build_ops(wk_sb, kr_sb, ec, t, costab, sintab):
                op()

        qr_tiles = {}

        def emit_qbuild(t, ec):
            ensure(("chain", t), lambda: emit_chain(t))
            if t not in qr_tiles:
                qr_tiles[t] = qrp.tile([P, ECH, QW], bf16, name=f"qr{t}")
            for op in build_ops(wq_sb, qr_tiles[t], ec, t, qcos, qsin):
                op()

        def emit_v(kc):
            # raw v projection, token-major (r_k rides the exp scale instead)
            vp = bldp.tile([P, QW], f32, space="PSUM", tag="bld", name="vp")
            for dc in range(DC):
                nc.tensor.matmul(
                    vp[:, 0:E],
                    xn_sb[:, kc // 4, dc, (kc % 4) * P:(kc % 4 + 1) * P],
                    wv_sb[:, dc, :],
                    start=(dc == 0), stop=(dc == DC - 1))
            vdst = v_sb[:, kc, :].rearrange("p (h c) -> p h c", c=65)[:, :, 0:64]
            nc.vector.tensor_scalar_mul(
                vdst, vp[:, 0:E].rearrange("p (h c) -> p h c", c=DH),
                r_tok[:, kc:kc + 1])

        def dep_round(pr, kc):
            tk = kc // 4
            ensure(("chain", tk), lambda: emit_chain(tk))
            ensure(("k", pr, tk), lambda: emit_kbuild(pr, tk))
            ensure(("v", kc), lambda: emit_v(kc))   # needs chain(tk) first

        lowpri = []

        def pops(n):
            for _ in range(n):
                if lowpri:
                    lowpri.pop(0)()

        # ---------------- attention + out-projection ----------------
        for t in range(TCH):
            tsl = slice(t * QW, (t + 1) * QW)
            af_t = attf.tile([P, ECH, QW], bf16, name=f"af{t}")

            for pr in range(NPAIR):
                hA, hB = 2 * pr, 2 * pr + 1
                ensure(("q", t, pr), lambda t=t, pr=pr: emit_qbuild(t, pr))
                qr_t = qr_tiles[t]
                dep_round(pr, 0)
                dep_round(pr, 1)
                avA = avp.tile([65, QW], f32, space="PSUM", tag="av", name="avA")
                avB = avp.tile([65, QW], f32, space="PSUM", tag="av", name="avB")
                sc_t = [None]
                pt_t = [None]

                def scores(kc):
                    s_ = scp.tile([P, 2 * QW], f32, space="PSUM", tag="sc",
                                  name="sc")
                    ksl = slice(kc * P, (kc + 1) * P)
                    nc.tensor.matmul(
                        s_[:, 0:QW], kr_sb[0:64, pr, ksl], qr_t[0:64, pr, :],
                        start=True, stop=True, tile_position=(0, 0))
                    nc.tensor.matmul(
                        s_[:, QW:2 * QW], kr_sb[64:128, pr, ksl],
                        qr_t[64:128, pr, :],
                        start=True, stop=True, tile_position=(64, 0))
                    p_ = ptp.tile([P, 2 * QW], bf16, name="pt")
                    nc.scalar.activation(p_[:], s_[:], AF.Exp,
                                         scale=rsc[:, kc:kc + 1])
                    sc_t[0], pt_t[0] = s_, p_

                def avmm(kc, p_):
                    nc.tensor.matmul(
                        avA[:], v_sb[:, kc, hA * 65:(hA + 1) * 65], p_[:, 0:QW],
                        start=(kc == 0), stop=(kc == KCH - 1))
                    nc.tensor.matmul(
                        avB[:], v_sb[:, kc, hB * 65:(hB + 1) * 65],
                        p_[:, QW:2 * QW],
                        start=(kc == 0), stop=(kc == KCH - 1))

                scores(0)
                for kc in range(KCH):
                    p_ = pt_t[0]
                    if kc < KCH - 1:
                        if kc + 2 < KCH:
                            dep_round(pr, kc + 2)
                        scores(kc + 1)
                    avmm(kc, p_)
                    pops(2)

                # normalize: batched rowsum reciprocal + DRAM broadcast
                rs = rsp.tile([1, 2 * QW], f32, tag="rs", name="rs")
                nc.vector.tensor_copy(out=rs[0:1, 0:QW], in_=avA[64:65, :])
                nc.vector.tensor_copy(out=rs[0:1, QW:2 * QW], in_=avB[64:65, :])
                rsr = rsp.tile([1, 2 * QW], f32, tag="rs", name="rsr")
                nc.vector.reciprocal_approx_fast(out=rsr[:], in_=rs[:])
                idx = t * NPAIR + pr
                nc.gpsimd.dma_start(out=scr_d[idx:idx + 1, :], in_=rsr[:])
                rep = repp.tile([P, QW], f32, name="rep")
                nc.gpsimd.dma_start(
                    out=rep[0:64, :],
                    in_=scr_d[idx:idx + 1, 0:QW].to_broadcast((64, QW)))
                nc.gpsimd.dma_start(
                    out=rep[64:128, :],
                    in_=scr_d[idx:idx + 1, QW:2 * QW].to_broadcast((64, QW)))
                nc.vector.tensor_mul(af_t[0:64, pr, :], avA[0:64, :], rep[0:64, :])
                nc.vector.tensor_mul(af_t[64:128, pr, :], avB[0:64, :],
                                     rep[64:128, :])

                # prefetch: next pair's k/q builds ride the low-pri queue
                if pr + 1 < NPAIR:
                    npr = pr + 1
                    lowpri.append(lambda npr=npr, t=t:
                                  ensure(("q", t, npr),
                                         lambda: emit_qbuild(t, npr)))
                    for tk in range(TCH):
                        lowpri.append(lambda npr=npr, tk=tk:
                                      ensure(("k", npr, tk),
                                             lambda: emit_kbuild(npr, tk)))
                elif t + 1 < TCH:
                    lowpri.append(lambda t=t: ensure(("q", t + 1, 0),
                                                     lambda: emit_qbuild(t + 1, 0)))

            # out-projection for this t rides the low-pri queue (overlaps t+1)
            def mk_outproj(af_c, tsl_c):
                ops = []
                for dmc in range(DC):
                    def op_mm(dmc=dmc):
                        st_ = avp.tile([P, QW], f32, space="PSUM", tag="av",
                                       name="opj")
                        outproj_ps[0] = st_
                        for ec in range(ECH):
                            nc.tensor.matmul(
                                st_[:], woT_sb[:, ec, dmc * P:(dmc + 1) * P],
                                af_c[:, ec, :], start=(ec == 0),
                                stop=(ec == ECH - 1))

                    def op_store(dmc=dmc):
                        og = ostg.tile([P, QW], f32, name="og")
                        nc.vector.tensor_copy(out=og[:], in_=outproj_ps[0][:])
                        nc.sync.dma_start(out=outT_r[:, dmc, tsl_c], in_=og[:])
                    ops.extend([op_mm, op_store])
                return ops

            lowpri.extend(mk_outproj(af_t, tsl))

        for op in lowpri:
            op()

    nc.compile()
    return nc


def _host_constants():
    j = np.arange(P) % 16
    invf = (ROPE_BASE ** (-(j / 16.0))).astype(np.float32).reshape(P, 1)
    A = np.zeros((P, P), np.float32)
    Bm = np.zeros((P, P), np.float32)
    for p in range(P):
        base = (p // 32) * 32
        jj = p % 32
        if jj < 16:
            A[p, base + 2 * jj] = 1.0
            Bm[p, base + 2 * jj + 1] = -1.0
        else:
            A[p, base + 2 * (jj - 16) + 1] = 1.0
            Bm[p, base + 2 * (jj - 16)] = 1.0
    return invf, np.ascontiguousarray(A.T), np.ascontiguousarray(Bm.T)


def _run(x, coords, ln_gamma, ln_beta, w_qkv, w_out, **run_kwargs):
    from concourse.bass_utils import run_bass_kernel_spmd

    x = np.asarray(x, np.float32)
    coords = np.asarray(coords, np.float32)
    ln_gamma = np.asarray(ln_gamma, np.float32)
    ln_beta = np.asarray(ln_beta, np.float32)
    w_qkv = np.asarray(w_qkv, np.float32)
    w_out = np.asarray(w_out, np.float32)
    assert not np.any(ln_beta != 0.0), "kernel assumes ln_beta == 0"

    if "g" not in _GRAPH_CACHE:
        _GRAPH_CACHE["g"] = _build_graph()
    nc = _GRAPH_CACHE["g"]

    invf, AT, BT = _host_constants()
    # fold ln_gamma, then center rows: (W - rowmean(W)) @ x == W @ (x - mu)
    wg = (w_qkv * ln_gamma[None, :]).astype(np.float32)
    wg = wg - wg.mean(axis=1, keepdims=True)
    wq, wk, wv = wg[0:D], wg[D:2 * D], wg[2 * D:3 * D]

    in_maps = []
    for core in range(8):
        b, g = core // 2, core % 2
        sl = slice(g * E, (g + 1) * E)
        m = {
            "xT": np.ascontiguousarray(
                x[b].T.reshape(DC, P, TCH, QW).transpose(2, 1, 0, 3)
            ).astype(ml_dtypes.bfloat16),
            "wqT": np.ascontiguousarray(
                wq[sl].T.reshape(DC, P, E).transpose(1, 0, 2)
            ).astype(ml_dtypes.bfloat16),
            "wkT": np.ascontiguousarray(
                wk[sl].T.reshape(DC, P, E).transpose(1, 0, 2)
            ).astype(ml_dtypes.bfloat16),
            "wvT": np.ascontiguousarray(
                wv[sl].T.reshape(DC, P, E).transpose(1, 0, 2)
            ).astype(ml_dtypes.bfloat16),
            "woT": np.ascontiguousarray(
                w_out[:, sl].T.reshape(ECH, P, D).transpose(1, 0, 2)
            ).astype(ml_dtypes.bfloat16),
            "coordsT": np.ascontiguousarray(coords[b].T),
            "invf": invf,
            "permA": AT,
            "permB": BT,
            "onesb": np.ones((P, 1), dtype=ml_dtypes.bfloat16),
        }
        in_maps.append(m)

    res = run_bass_kernel_spmd(nc, in_maps, core_ids=list(range(8)), **run_kwargs)
    out = np.empty((B, N, D), np.float32)
    for b in range(B):
        acc = res.results[2 * b]["out"] + res.results[2 * b + 1]["out"]
        out[b] = acc.T
    return out, res


def kernel(x, coords, ln_gamma, ln_beta, w_qkv, w_out):
    out, _ = _run(x, coords, ln_gamma, ln_beta, w_qkv, w_out)
    return out


# revision 41
# speedup vs baseline: 1.2432x; 1.2432x over previous
"""Trainium2 Bass kernel for nn_Attention_60576218743412.

LayerNorm -> QKV projection -> 2D axial RoPE -> full softmax attention ->
out-projection, for x[B=4, N=2048, D=768], 12 heads of 64.

Sharding: 8 cores = 4 batches x 2 head-groups (6 heads each).  Each core
computes LN + QKV for its 6 heads, attention, and a partial out-projection
(its 384 columns of w_out); the host sums the two partials per batch.

v4 structure (from trace analysis of v1-v3):
- LN mean-subtraction is folded into host-side weight row-centering, so all
  projections run on RAW bf16 x.
- The LN scale r[t] is applied three ways so almost nothing waits on stats:
  k/v are built r-FREE from plain cos/sin tables; r_k rides the exp as a
  per-partition scale AP (and 1/r_k sits in the v "ones" column so softmax
  denominators stay unscaled); r_q is folded into per-t q-side tables.
- Attention is interleaved with the prelude: per (pr, t) the 16 key-chunk
  rounds ensure their own deps (per-t stats chain, k-build, v-build) on
  demand two rounds ahead, so exp starts as soon as x(t0) lands instead of
  after the full prelude.
- Feature-major AV with the 65th ones-column rowsum; normalize via
  reciprocal_approx_fast + DRAM-broadcast; out-projection woven into the
  next token-chunk's rounds.
"""

import numpy as np
import ml_dtypes

B, N, D = 4, 2048, 768
HEADS, DH = 12, 64
HG = 6                # heads per core
E = HG * DH           # 384: per-core qkv width
ROPE_BASE = 8192.0
LN_EPS = 1e-5
P = 128
DC = D // P           # 6 contraction chunks
ECH = E // P          # 3 e-chunks
TCH = 4               # token chunks for 512-wide matmuls
QW = N // TCH         # 512
KCH = N // P          # 16 key chunks
NPAIR = HG // 2       # head pairs per core

_GRAPH_CACHE = {}


def _build_graph():
    from contextlib import ExitStack

    import concourse.tile as tile
    from concourse import bacc, mybir

    f32 = mybir.dt.float32
    f32r = mybir.dt.float32r
    bf16 = mybir.dt.bfloat16
    AL = mybir.AluOpType
    AF = mybir.ActivationFunctionType

    nc = bacc.Bacc(None, target_bir_lowering=False)

    xT = nc.dram_tensor("xT", [TCH, P, DC, QW], bf16, kind="ExternalInput")
    wqT = nc.dram_tensor("wqT", [P, DC, E], bf16, kind="ExternalInput")
    wkT = nc.dram_tensor("wkT", [P, DC, E], bf16, kind="ExternalInput")
    wvT = nc.dram_tensor("wvT", [P, DC, E], bf16, kind="ExternalInput")
    woT = nc.dram_tensor("woT", [P, ECH, D], bf16, kind="ExternalInput")
    coordsT = nc.dram_tensor("coordsT", [2, N], f32, kind="ExternalInput")
    invf = nc.dram_tensor("invf", [P, 1], f32, kind="ExternalInput")
    permA = nc.dram_tensor("permA", [P, P], f32r, kind="ExternalInput")
    permB = nc.dram_tensor("permB", [P, P], f32r, kind="ExternalInput")
    onesb = nc.dram_tensor("onesb", [P, 1], bf16, kind="ExternalInput")
    outT = nc.dram_tensor("out", [D, N], f32, kind="ExternalOutput")

    outT_r = outT.rearrange("(c p) t -> p c t", p=P)

    MAGIC = float(2.0 ** 23)
    TWO_PI = float(2 * np.pi)
    SCALE = float(DH ** -0.5)

    with tile.TileContext(nc) as tc, ExitStack() as octx:
        consts = octx.enter_context(tc.tile_pool(name="consts", bufs=1))
        persist = octx.enter_context(tc.tile_pool(name="persist", bufs=1))
        dram = octx.enter_context(tc.tile_pool(name="dram", bufs=1, space="DRAM"))

        # PSUM: sc 2 banks x2 + av 1 bank x4 = 8 banks
        scp = octx.enter_context(tc.tile_pool(name="sc_ps", bufs=2, space="PSUM"))
        avp = octx.enter_context(tc.tile_pool(name="av_ps", bufs=4, space="PSUM"))
        outproj_ps = [None]

        # ---------------- constants ----------------
        invf_sb = consts.tile([P, 1], f32)
        nc.scalar.dma_start(out=invf_sb[:], in_=invf[:])
        pA_sb = consts.tile([P, P], f32r)
        nc.scalar.dma_start(out=pA_sb[:], in_=permA[:])
        pB_sb = consts.tile([P, P], f32r)
        nc.scalar.dma_start(out=pB_sb[:], in_=permB[:])
        woT_sb = consts.tile([P, ECH, D], bf16)
        nc.scalar.dma_start(out=woT_sb[:], in_=woT[:])
        pi2_sb = consts.tile([P, 1], f32)
        nc.vector.memset(pi2_sb[:], float(np.pi / 2))
        onesb_sb = consts.tile([P, 1], bf16)
        nc.scalar.dma_start(out=onesb_sb[:], in_=onesb[:])

        # persistent state
        xn_sb = persist.tile([P, TCH, DC, QW], bf16)  # raw x, [t, dc, q]
        wq_sb = persist.tile([P, DC, E], bf16, tag="wq")
        wk_sb = persist.tile([P, DC, E], bf16, tag="wk")
        nc.gpsimd.dma_start(out=wk_sb[:], in_=wkT[:])
        wv_sb = persist.tile([P, DC, E], bf16, tag="wv")
        nc.gpsimd.dma_start(out=wv_sb[:], in_=wvT[:])
        kr_sb = persist.tile([P, ECH, N], bf16)     # rotated k (r-free)
        v_sb = persist.tile([P, KCH, HG * 65], bf16)  # raw v | 1/r_k col
        costab = persist.tile([P, N], f32)          # plain cos
        sintab = persist.tile([P, N], f32)          # plain sin
        qcos = persist.tile([P, N], f32)            # r_q * cos (per-t filled)
        qsin = persist.tile([P, N], f32)            # r_q * sin
        r_tok = persist.tile([P, KCH], f32)         # r, token-major
        rsc = persist.tile([P, KCH], f32)           # r * dh^-0.5 (exp scale)

        sums_d = dram.tile([TCH, 2 * QW], f32)   # per t: [sum | sumsq]
        r_d = dram.tile([1, N], f32)
        scr_d = dram.tile([TCH * NPAIR, 2 * QW], f32)

        # ones columns of v (become 1/r_k once stats land)
        for h in range(HG):
            nc.gpsimd.dma_start(
                out=v_sb[:, :, h * 65 + 64: h * 65 + 65],
                in_=onesb[:, 0:1][:, :, None].to_broadcast((P, KCH, 1)))

        # ---------------- RoPE trig tables (plain) ----------------
        with ExitStack() as ptab:
            tblp = ptab.enter_context(tc.tile_pool(name="tbl", bufs=1))
            ftab = tblp.tile([P, N], f32, name="ftab")
            for blk in range(4):
                axis = blk % 2
                nc.sync.dma_start(
                    out=ftab[32 * blk: 32 * blk + 32, :],
                    in_=coordsT[axis: axis + 1, :].to_broadcast((32, N)),
                )
            nc.vector.tensor_scalar_mul(ftab[:], ftab[:], invf_sb[:])
            # round-to-nearest via +-2^23; costab doubles as the scratch
            nc.vector.tensor_scalar(
                costab[:], ftab[:], 1.0 / TWO_PI, MAGIC, AL.mult, AL.add)
            nc.vector.tensor_scalar_sub(costab[:], costab[:], MAGIC)
            nc.vector.scalar_tensor_tensor(
                sintab[:], costab[:], -TWO_PI, ftab[:], AL.mult, AL.add)
            nc.scalar.activation(sintab[:], sintab[:], AF.Sin)
            nc.vector.tensor_scalar(
                costab[:], ftab[:], 1.0 / TWO_PI, 0.25, AL.mult, AL.add)
            nc.vector.tensor_scalar_add(costab[:], costab[:], MAGIC)
            nc.vector.tensor_scalar_sub(costab[:], costab[:], MAGIC)
            nc.vector.scalar_tensor_tensor(
                costab[:], costab[:], -TWO_PI, ftab[:], AL.mult, AL.add)
            nc.scalar.activation(costab[:], costab[:], AF.Sin, bias=pi2_sb[:])

        # working pools (created after the table scratch is released)
        rawp = octx.enter_context(tc.tile_pool(name="raw", bufs=2))
        cmbp = octx.enter_context(tc.tile_pool(name="cmb", bufs=1))
        ptp = octx.enter_context(tc.tile_pool(name="pt", bufs=2))
        qrp = octx.enter_context(tc.tile_pool(name="qr", bufs=2))
        attf = octx.enter_context(tc.tile_pool(name="attf", bufs=2))
        rsp = octx.enter_context(tc.tile_pool(name="rsp", bufs=2))
        repp = octx.enter_context(tc.tile_pool(name="repp", bufs=2))
        ostg = octx.enter_context(tc.tile_pool(name="ostg", bufs=2))
        rrep = octx.enter_context(tc.tile_pool(name="rrep", bufs=1))
        sqp = octx.enter_context(tc.tile_pool(name="xsq", bufs=2))
        stcp = octx.enter_context(tc.tile_pool(name="stc", bufs=1))
        smallp = octx.enter_context(tc.tile_pool(name="small", bufs=1))

        # x loads; wq last on sync (needed latest)
        for t, eng in zip(range(TCH), (nc.sync, nc.gpsimd, nc.sync, nc.gpsimd)):
            eng.dma_start(out=xn_sb[:, t], in_=xT[t])
        nc.sync.dma_start(out=wq_sb[:], in_=wqT[:])

        stT = smallp.tile([P, 2, KCH], f32)
        mu = smallp.tile([P, KCH], f32)
        var = smallp.tile([P, KCH], f32)
        musq = smallp.tile([P, KCH], f32)
        sdev = smallp.tile([P, KCH], f32)

        # ---------------- on-demand emission machinery ----------------
        done = set()

        def ensure(key, fn):
            if key not in done:
                done.add(key)
                fn()

        def emit_stats(t):
            # LN sums for token chunk t (feature-major ones-matmuls)
            st = scp.tile([P, 2 * QW], f32, space="PSUM", tag="sc", name="st")
            for dc in range(DC):
                xsq = sqp.tile([P, QW], bf16, name="xsq")
                nc.vector.tensor_mul(xsq[:], xn_sb[:, t, dc, :], xn_sb[:, t, dc, :])
                nc.tensor.matmul(
                    st[0:1, 0:QW], onesb_sb[:], xn_sb[:, t, dc, :],
                    start=(dc == 0), stop=(dc == DC - 1))
                nc.tensor.matmul(
                    st[0:1, QW:2 * QW], onesb_sb[:], xsq[:],
                    start=(dc == 0), stop=(dc == DC - 1))
            stc = stcp.tile([1, 2 * QW], f32, name="stc")
            nc.vector.tensor_copy(out=stc[:], in_=st[0:1, :])
            nc.scalar.dma_start(out=sums_d[t:t + 1, :], in_=stc[:])

        def emit_chain(t):
            # stats -> r(t) -> exp-scale rsc, q-tables, v 1/r columns for t
            ensure(("stats", t), lambda: emit_stats(t))
            ksl4 = slice(t * 4, (t + 1) * 4)
            for s in range(2):
                nc.scalar.dma_start(
                    out=stT[:, s, ksl4],
                    in_=sums_d[t:t + 1, s * QW:(s + 1) * QW].rearrange(
                        "o (c p) -> p (o c)", p=P))
            nc.vector.tensor_scalar_mul(mu[:, ksl4], stT[:, 0, ksl4], 1.0 / D)
            nc.vector.tensor_scalar(
                var[:, ksl4], stT[:, 1, ksl4], 1.0 / D, float(LN_EPS),
                AL.mult, AL.add)
            nc.vector.tensor_mul(musq[:, ksl4], mu[:, ksl4], mu[:, ksl4])
            nc.vector.tensor_sub(var[:, ksl4], var[:, ksl4], musq[:, ksl4])
            nc.scalar.activation(sdev[:, ksl4], var[:, ksl4], AF.Sqrt)
            nc.vector.reciprocal(r_tok[:, ksl4], sdev[:, ksl4])
            nc.vector.tensor_scalar_mul(rsc[:, ksl4], r_tok[:, ksl4], SCALE)
            # q-side tables for chunk t: qcos = r * cos, qsin = r * sin
            tsl = slice(t * QW, (t + 1) * QW)
            nc.scalar.dma_start(
                out=r_d[0:1, tsl].rearrange("o (c p) -> p (o c)", p=P),
                in_=r_tok[:, ksl4])
            rr = rrep.tile([P, QW], f32, name="rr")
            nc.scalar.dma_start(out=rr[:], in_=r_d[0:1, tsl].to_broadcast((P, QW)))
            nc.vector.tensor_mul(qcos[:, tsl], costab[:, tsl], rr[:])
            nc.vector.tensor_mul(qsin[:, tsl], sintab[:, tsl], rr[:])

        def build_ops(w_sb, dst, ec, t, ctab, stab):
            """Coarse micro-ops for one projected+rotated [128, QW] tile."""
            tsl = slice(t * QW, (t + 1) * QW)
            esl = slice(ec * P, (ec + 1) * P)
            state = {}

            def op_proj():
                state["pj"] = scp.tile([P, 2 * QW], f32, space="PSUM",
                                       tag="sc", name="pj")
                for dc in range(DC):
                    nc.tensor.matmul(
                        state["pj"][:, 0:QW], w_sb[:, dc, esl], xn_sb[:, t, dc, :],
                        start=(dc == 0), stop=(dc == DC - 1))

            def op_raw():
                state["raw"] = rawp.tile([P, QW], f32r, name="raw")
                nc.vector.tensor_copy(out=state["raw"][:], in_=state["pj"][:, 0:QW])

            def op_ep():
                state["pp"] = scp.tile([P, 2 * QW], f32, space="PSUM",
                                       tag="sc", name="pp")
                nc.tensor.matmul(state["pp"][:, 0:QW], pA_sb[:], state["raw"][:],
                                 start=True, stop=True)

            def op_t1():
                state["t1"] = cmbp.tile([P, QW], f32, tag="t1", name="t1")
                nc.vector.tensor_mul(state["t1"][:], state["pp"][:, 0:QW],
                                     ctab[:, tsl])

            def op_op():
                nc.tensor.matmul(state["pp"][:, QW:2 * QW], pB_sb[:],
                                 state["raw"][:], start=True, stop=True)

            def op_t2():
                state["t2"] = cmbp.tile([P, QW], f32, tag="t2", name="t2")
                nc.vector.tensor_mul(state["t2"][:], state["pp"][:, QW:2 * QW],
                                     stab[:, tsl])

            def op_add():
                nc.gpsimd.tensor_add(dst[:, ec, :] if dst.shape[2] == QW
                                     else dst[:, ec, tsl],
                                     state["t1"][:], state["t2"][:])

            return [op_proj, op_raw, op_ep, op_t1, op_op, op_t2, op_add]

        def emit_kbuild(ec, t):
            for op in build_ops(wk_sb, kr_sb, ec, t, costab, sintab):
                op()

        qr_tiles = {}

        def emit_qbuild(t, ec):
            ensure(("chain", t), lambda: emit_chain(t))
            if t not in qr_tiles:
                qr_tiles[t] = qrp.tile([P, ECH, QW], bf16, name=f"qr{t}")
            for op in build_ops(wq_sb, qr_tiles[t], ec, t, qcos, qsin):
                op()

        def emit_v(kc):
            # raw v projection, token-major (r_k rides the exp scale instead)
            vp = scp.tile([P, 2 * QW], f32, space="PSUM", tag="sc", name="vp")
            for dc in range(DC):
                nc.tensor.matmul(
                    vp[:, 0:E],
                    xn_sb[:, kc // 4, dc, (kc % 4) * P:(kc % 4 + 1) * P],
                    wv_sb[:, dc, :],
                    start=(dc == 0), stop=(dc == DC - 1))
            vdst = v_sb[:, kc, :].rearrange("p (h c) -> p h c", c=65)[:, :, 0:64]
            nc.vector.tensor_scalar_mul(
                vdst, vp[:, 0:E].rearrange("p (h c) -> p h c", c=DH),
                r_tok[:, kc:kc + 1])

        lowpri = []

        def pops(n):
            for _ in range(n):
                if lowpri:
                    lowpri.pop(0)()

        # ---------------- monolithic prelude ----------------
        # stats first (PE + DVE), then the r-chains (DVE/ACT/DMA only --
        # they overlap the r-free k-builds on PE), then v (needs r), then q(t0)
        for t in range(TCH):
            ensure(("stats", t), lambda t=t: emit_stats(t))
        for t in range(TCH):
            ensure(("chain", t), lambda t=t: emit_chain(t))
        for ec in range(ECH):
            for t in range(TCH):
                ensure(("k", ec, t), lambda ec=ec, t=t: emit_kbuild(ec, t))
        for kc in range(KCH):
            ensure(("v", kc), lambda kc=kc: emit_v(kc))
        for ec in range(ECH):
            ensure(("q", 0, ec), lambda ec=ec: emit_qbuild(0, ec))

        # ---------------- attention + out-projection ----------------
        for t in range(TCH):
            tsl = slice(t * QW, (t + 1) * QW)
            af_t = attf.tile([P, ECH, QW], bf16, name=f"af{t}")
            if t + 1 < TCH:
                for ec in range(ECH):
                    def q_ops(t=t, ec=ec):
                        return lambda: ensure(
                            ("q", t + 1, ec), lambda: emit_qbuild(t + 1, ec))
                    lowpri.append(q_ops())

            for pr in range(NPAIR):
                hA, hB = 2 * pr, 2 * pr + 1
                qr_t = qr_tiles[t]
                avA = avp.tile([65, QW], f32, space="PSUM", tag="av", name="avA")
                avB = avp.tile([65, QW], f32, space="PSUM", tag="av", name="avB")
                sc_t = [None]
                pt_t = [None]

                def scores(kc):
                    s_ = scp.tile([P, 2 * QW], f32, space="PSUM", tag="sc",
                                  name="sc")
                    ksl = slice(kc * P, (kc + 1) * P)
                    nc.tensor.matmul(
                        s_[:, 0:QW], kr_sb[0:64, pr, ksl], qr_t[0:64, pr, :],
                        start=True, stop=True, tile_position=(0, 0))
                    nc.tensor.matmul(
                        s_[:, QW:2 * QW], kr_sb[64:128, pr, ksl],
                        qr_t[64:128, pr, :],
                        start=True, stop=True, tile_position=(64, 0))
                    p_ = ptp.tile([P, 2 * QW], bf16, name="pt")
                    nc.scalar.activation(p_[:], s_[:], AF.Exp,
                                         scale=rsc[:, kc:kc + 1])
                    sc_t[0], pt_t[0] = s_, p_

                def avmm(kc, p_):
                    nc.tensor.matmul(
                        avA[:], v_sb[:, kc, hA * 65:(hA + 1) * 65], p_[:, 0:QW],
                        start=(kc == 0), stop=(kc == KCH - 1))
                    nc.tensor.matmul(
                        avB[:], v_sb[:, kc, hB * 65:(hB + 1) * 65],
                        p_[:, QW:2 * QW],
                        start=(kc == 0), stop=(kc == KCH - 1))

                scores(0)
                for kc in range(KCH):
                    p_ = pt_t[0]
                    if kc < KCH - 1:
                        scores(kc + 1)
                    avmm(kc, p_)
                    pops(2)

                # normalize: batched rowsum reciprocal + DRAM broadcast
                rs = rsp.tile([1, 2 * QW], f32, tag="rs", name="rs")
                nc.vector.tensor_copy(out=rs[0:1, 0:QW], in_=avA[64:65, :])
                nc.vector.tensor_copy(out=rs[0:1, QW:2 * QW], in_=avB[64:65, :])
                rsr = rsp.tile([1, 2 * QW], f32, tag="rs", name="rsr")
                nc.vector.reciprocal_approx_fast(out=rsr[:], in_=rs[:])
                idx = t * NPAIR + pr
                nc.gpsimd.dma_start(out=scr_d[idx:idx + 1, :], in_=rsr[:])
                rep = repp.tile([P, QW], f32, name="rep")
                nc.gpsimd.dma_start(
                    out=rep[0:64, :],
                    in_=scr_d[idx:idx + 1, 0:QW].to_broadcast((64, QW)))
                nc.gpsimd.dma_start(
                    out=rep[64:128, :],
                    in_=scr_d[idx:idx + 1, QW:2 * QW].to_broadcast((64, QW)))
                nc.vector.tensor_mul(af_t[0:64, pr, :], avA[0:64, :], rep[0:64, :])
                nc.vector.tensor_mul(af_t[64:128, pr, :], avB[0:64, :],
                                     rep[64:128, :])

            # out-projection for this t rides the low-pri queue (overlaps t+1)
            def mk_outproj(af_c, tsl_c):
                ops = []
                for dmc in range(DC):
                    def op_mm(dmc=dmc):
                        st_ = avp.tile([P, QW], f32, space="PSUM", tag="av",
                                       name="opj")
                        outproj_ps[0] = st_
                        for ec in range(ECH):
                            nc.tensor.matmul(
                                st_[:], woT_sb[:, ec, dmc * P:(dmc + 1) * P],
                                af_c[:, ec, :], start=(ec == 0),
                                stop=(ec == ECH - 1))

                    def op_store(dmc=dmc):
                        og = ostg.tile([P, QW], f32, name="og")
                        nc.vector.tensor_copy(out=og[:], in_=outproj_ps[0][:])
                        nc.sync.dma_start(out=outT_r[:, dmc, tsl_c], in_=og[:])
                    ops.extend([op_mm, op_store])
                return ops

            lowpri.extend(mk_outproj(af_t, tsl))

        for op in lowpri:

            op()

    nc.compile()
    return nc


def _host_constants():
    j = np.arange(P) % 16
    invf = (ROPE_BASE ** (-(j / 16.0))).astype(np.float32).reshape(P, 1)
    A = np.zeros((P, P), np.float32)
    Bm = np.zeros((P, P), np.float32)
    for p in range(P):
        base = (p // 32) * 32
        jj = p % 32
        if jj < 16:
            A[p, base + 2 * jj] = 1.0
            Bm[p, base + 2 * jj + 1] = -1.0
        else:
            A[p, base + 2 * (jj - 16) + 1] = 1.0
            Bm[p, base + 2 * (jj - 16)] = 1.0
    return invf, np.ascontiguousarray(A.T), np.ascontiguousarray(Bm.T)


def _run(x, coords, ln_gamma, ln_beta, w_qkv, w_out, **run_kwargs):
    from concourse.bass_utils import run_bass_kernel_spmd

    x = np.asarray(x, np.float32)
    coords = np.asarray(coords, np.float32)
    ln_gamma = np.asarray(ln_gamma, np.float32)
    ln_beta = np.asarray(ln_beta, np.float32)
    w_qkv = np.asarray(w_qkv, np.float32)
    w_out = np.asarray(w_out, np.float32)
    assert not np.any(ln_beta != 0.0), "kernel assumes ln_beta == 0"

    if "g" not in _GRAPH_CACHE:
        _GRAPH_CACHE["g"] = _build_graph()
    nc = _GRAPH_CACHE["g"]

    invf, AT, BT = _host_constants()
    # fold ln_gamma, then center rows: (W - rowmean(W)) @ x == W @ (x - mu)
    wg = (w_qkv * ln_gamma[None, :]).astype(np.float32)
    wg = wg - wg.mean(axis=1, keepdims=True)
    wq, wk, wv = wg[0:D], wg[D:2 * D], wg[2 * D:3 * D]

    in_maps = []
    for core in range(8):
        b, g = core // 2, core % 2
        sl = slice(g * E, (g + 1) * E)
        m = {
            "xT": np.ascontiguousarray(
                x[b].T.reshape(DC, P, TCH, QW).transpose(2, 1, 0, 3)
            ).astype(ml_dtypes.bfloat16),
            "wqT": np.ascontiguousarray(
                wq[sl].T.reshape(DC, P, E).transpose(1, 0, 2)
            ).astype(ml_dtypes.bfloat16),
            "wkT": np.ascontiguousarray(
                wk[sl].T.reshape(DC, P, E).transpose(1, 0, 2)
            ).astype(ml_dtypes.bfloat16),
            "wvT": np.ascontiguousarray(
                wv[sl].T.reshape(DC, P, E).transpose(1, 0, 2)
            ).astype(ml_dtypes.bfloat16),
            "woT": np.ascontiguousarray(
                w_out[:, sl].T.reshape(ECH, P, D).transpose(1, 0, 2)
            ).astype(ml_dtypes.bfloat16),
            "coordsT": np.ascontiguousarray(coords[b].T),
            "invf": invf,
            "permA": AT,
            "permB": BT,
            "onesb": np.ones((P, 1), dtype=ml_dtypes.bfloat16),
        }
        in_maps.append(m)

    res = run_bass_kernel_spmd(nc, in_maps, core_ids=list(range(8)), **run_kwargs)
    out = np.empty((B, N, D), np.float32)
    for b in range(B):
        acc = res.results[2 * b]["out"] + res.results[2 * b + 1]["out"]
        out[b] = acc.T
    return out, res


def kernel(x, coords, ln_gamma, ln_beta, w_qkv, w_out):
    out, _ = _run(x, coords, ln_gamma, ln_beta, w_qkv, w_out)
    return out
